# revision 21
# baseline (speedup 1.0000x reference)
"""Trainium2 Bass kernel for the Contextualizer odd-layer block.

Computes, for x [B=4, T=2048, D=2048]:
    x0, x1 = split(x, 2, axis=-1)                 # [B, T, d], d=1024
    u[t]   = 1/sqrt(sum(x0[t]^2) + eps)
    sim    = (normed @ normed^T)                  # cosine similarity, TxT
    sim   /= (row_sums + eps)
    out    = (sim @ x0) * x1

Sharding: 8 cores = 4 batches x 2 query-halves. Each core gets the full
x0 of its batch with rows permuted so its own 1024 query rows come first
(key/value sums are permutation-invariant), plus its x1 slice.

Per-core dataflow (q = rows 0:1024 of the permuted x0):
    ss[t]  = sum_d x0[t,d]^2          (ACT Square + accum_out)
    u[t]   = 1/sqrt(ss + eps)         (ACT Sqrt + DVE reciprocal)
    x0T    = transpose(x0)            (PE transpose -> f32r, [d, T] layout)
    G^T    = x0T[:,s_tile]^T x0T[:,q] (PE, f32r)     # [s, q] blocks
    P      = u[s] * G^T               (ACT copy w/ per-partition scale)
    mix    = P^T @ x0                 (PE, f32r)     # [q, d] numerator
    v_acc  = sum_tiles u*x0           (DVE accumulate, fp32, per-partition)
    v      = ones^T @ v_acc           (PE fp32)      # partition-sum
    w[q]   = x0[q,:] . v              (DVE STT+accum vs broadcast v, fp32)
    fscale = u[q] / (u[q]*w[q] + eps)
    out    = (mix * fscale) * x1      (DVE scalar_tensor_tensor)

The row-sum denominator rs = u*w is a near-cancelling sum, so it is kept
on an fp32-exact path (f32r rounds to ~12 mantissa bits, which blows up
rows where rs+eps is tiny). The numerator tolerates f32r.

Phase A (load/norm/transpose, DMA-bound) is software-pipelined with the
first sim-matmul block: 4 tiles are prefilled, then each loop step emits
matmul1(s-tile st) followed by the staging of tile st+4, so the PE picks
up sim matmuls ~12us in instead of waiting for the whole 8.4MB load.
"""

import numpy as np

T = 2048
D_HALF = 1024
NT = T // 128          # 16 s-tiles
NQ = D_HALF // 128     # 8 q-tiles per core
EPS = 1e-6
N_CORES = 8
PREFILL = 4

_CACHE = {}


def _build_nc():
    import concourse.bacc as bacc
    import concourse.mybir as mybir
    import concourse.tile as tile

    F32 = mybir.dt.float32
    F32R = mybir.dt.float32r
    AF = mybir.ActivationFunctionType
    OP = mybir.AluOpType

    nc = bacc.Bacc("TRN2", target_bir_lowering=False, debug=False)
    x0_d = nc.dram_tensor("x0", [T, D_HALF], F32, kind="ExternalInput").ap()
    x1_d = nc.dram_tensor("x1", [D_HALF, D_HALF], F32, kind="ExternalInput").ap()
    id_d = nc.dram_tensor("ident", [128, 128], F32, kind="ExternalInput").ap()
    out_d = nc.dram_tensor("out", [D_HALF, D_HALF], F32, kind="ExternalOutput").ap()

    with tile.TileContext(nc) as tc:
        with (
            tc.tile_pool(name="persist", bufs=1) as persist,
            tc.tile_pool(name="stream", bufs=2) as stream,
            tc.tile_pool(name="psum", bufs=2, space="PSUM") as psum,
        ):
            ident = persist.tile([128, 128], F32, name="ident")
            nc.sync.dma_start(ident[:], id_d[:])
            ident_r = persist.tile([128, 128], F32R, name="ident_r")
            nc.vector.tensor_copy(ident_r[:], ident[:])
            eps_t = persist.tile([128, 1], F32, name="eps_t")
            nc.vector.memset(eps_t[:], EPS)
            ones_f = persist.tile([128, 1], F32, name="ones_f")
            nc.vector.memset(ones_f[:], 1.0)

            # x0T [d, T] in f32r: free index = dc*T + t for d-chunk dc
            x0T = persist.tile([128, 8 * T], F32R, name="x0T")
            x0T3 = x0T.rearrange("p (c t) -> p c t", c=8)
            v_acc = persist.tile([128, D_HALF], F32, name="v_acc")

            x0r = [None] * NT   # f32r natural-layout values (matmul2 rhs)
            us = [None] * NT    # 1/norm per-partition scalars

            deferred_v = []   # (st, halves) for tiles whose v_acc update is late

            def v_update(st, halves):
                """fp32 per-partition accumulation of u[s]*x0[s,:] on DVE."""
                if st == 0:
                    nc.vector.tensor_scalar_mul(v_acc[:], halves[0][:], us[st][:])
                else:
                    nc.vector.scalar_tensor_tensor(
                        v_acc[:], halves[0][:], us[st][:], v_acc[:],
                        op0=OP.mult, op1=OP.add,
                    )

            def stage_tile(st, defer_v=False):
                """DMA tile st, compute u, f32r copy, v accumulation, transpose."""
                rows = slice(st * 128, (st + 1) * 128)
                x0n = stream.tile([128, D_HALF], F32, name="x0n", tag="x0n", bufs=3)
                dma_eng = nc.sync if st % 2 == 0 else nc.scalar
                dma_eng.dma_start(x0n[:], x0_d[rows, :])

                sq = stream.tile([128, D_HALF], F32, name="sq", tag="sq", bufs=1)
                ss = stream.tile([128, 1], F32, name="ss", tag="ss", bufs=4)
                nc.scalar.activation(sq[:], x0n[:], AF.Square, accum_out=ss[:])
                nrm = stream.tile([128, 1], F32, name="nrm", tag="nrm", bufs=4)
                nc.scalar.activation(nrm[:], ss[:], AF.Sqrt, bias=eps_t[:])
                u = persist.tile([128, 1], F32, name=f"u{st}")
                nc.vector.reciprocal(u[:], nrm[:])
                us[st] = u

                xr = persist.tile([128, D_HALF], F32R, name=f"x0r{st}")
                nc.vector.tensor_copy(xr[:], x0n[:])
                x0r[st] = xr

                for dg in range(2):
                    tps = psum.tile([128, 512], F32R, name="tps", tag="tps")
                    for k in range(4):
                        dc = dg * 4 + k
                        nc.tensor.transpose(
                            tps[:, k * 128:(k + 1) * 128],
                            xr[:, dc * 128:(dc + 1) * 128],
                            ident_r[:],
                        )
                    src = tps.rearrange("p (c t) -> p c t", c=4)
                    nc.vector.tensor_copy(
                        x0T3[:, dg * 4:(dg + 1) * 4, rows],
                        src[:],
                    )

                if defer_v:
                    deferred_v.append((st, [x0n]))
                else:
                    v_update(st, [x0n])

            def matmul1(qb, st, Ps):
                """sim block: P[st][:, qb] = u[st] * (x0T[:,st]^T @ x0T[:,qb])."""
                ps_sim = psum.tile([128, 512], F32, name="ps_sim", tag="sim")
                for dc in range(8):
                    nc.tensor.matmul(
                        ps_sim[:],
                        x0T3[:, dc, st * 128:(st + 1) * 128],
                        x0T3[:, dc, qb * 512:(qb + 1) * 512],
                        start=(dc == 0),
                        stop=(dc == 7),
                    )
                P = persist.tile([128, 512], F32R, name=f"P{st}", tag=f"P{st}")
                nc.scalar.mul(P[:], ps_sim[:], us[st][:])
                Ps.append(P)

            def matmul2(qb, qt, Ps, ws):
                """mix = P^T @ x0, scaled and gated -> out rows."""
                qg = qb * 4 + qt
                x1t = stream.tile([128, D_HALF], F32, name="x1t", tag="x1t")
                nc.sync.dma_start(x1t[:], x1_d[qg * 128:(qg + 1) * 128, :])

                # fscale = u_q / (u_q * w + eps)
                den = stream.tile([128, 1], F32, name="den", tag="den")
                nc.vector.scalar_tensor_tensor(
                    den[:], ws[qg][:], us[qg][:], eps_t[:],
                    op0=OP.mult, op1=OP.add,
                )
                rec = stream.tile([128, 1], F32, name="rec", tag="rec")
                nc.vector.reciprocal(rec[:], den[:])
                fscale = stream.tile([128, 1], F32, name="fscale", tag="fscale")
                nc.vector.tensor_scalar_mul(fscale[:], rec[:], us[qg][:])

                out_sb = stream.tile([128, D_HALF], F32, name="out_sb", tag="out_sb")
                for half in range(2):
                    cols = slice(half * 512, (half + 1) * 512)
                    ps_mix = psum.tile([128, 512], F32, name="ps_mix", tag="mix")
                    for st in range(NT):
                        nc.tensor.matmul(
                            ps_mix[:],
                            Ps[st][:, qt * 128:(qt + 1) * 128],
                            x0r[st][:, cols],
                            start=(st == 0),
                            stop=(st == NT - 1),
                        )
                    nc.vector.scalar_tensor_tensor(
                        out_sb[:, cols], ps_mix[:], fscale[:], x1t[:, cols],
                        op0=OP.mult, op1=OP.mult,
                    )
                    nc.sync.dma_start(
                        out_d[qg * 128:(qg + 1) * 128, cols], out_sb[:, cols])

            # ---- phase A prefill + pipelined B(qb=0) ----
            for st in range(PREFILL):
                stage_tile(st)
            Ps0 = []
            for st in range(NT):
                matmul1(0, st, Ps0)
                if st + PREFILL < NT:
                    stage_tile(st + PREFILL, defer_v=(st + PREFILL >= NT - 2))

            for st, halves in deferred_v:
                v_update(st, halves)

            # ---- v partition-sum on GPSIMD (keeps PE out of the w chain) ----
            import concourse.bass_isa as bass_isa
            v_bc = persist.tile([128, D_HALF], F32, name="v_bc")
            nc.gpsimd.partition_all_reduce(
                v_bc[:], v_acc[:], channels=128, reduce_op=bass_isa.ReduceOp.add,
            )

            ws = []
            for qt in range(NQ):
                xq = stream.tile([128, D_HALF], F32, name="xq", tag="x0n", bufs=3)
                dma_eng = nc.sync if qt % 2 == 0 else nc.scalar
                dma_eng.dma_start(xq[:], x0_d[qt * 128:(qt + 1) * 128, :])
                wscr = stream.tile([128, D_HALF], F32, name="wscr", tag="sq", bufs=1)
                w_sb = persist.tile([128, 1], F32, name=f"w{qt}")
                nc.vector.scalar_tensor_tensor(
                    wscr[:], xq[:], 1.0, v_bc[:],
                    op0=OP.mult, op1=OP.mult, accum_out=w_sb[:],
                )
                ws.append(w_sb)

            # ---- C(qb=0), then B(qb=1) + C(qb=1) ----
            for qt in range(4):
                matmul2(0, qt, Ps0, ws)
            Ps1 = []
            for st in range(NT):
                matmul1(1, st, Ps1)
            for qt in range(4):
                matmul2(1, qt, Ps1, ws)

    nc.compile()
    return nc


def _get_nc():
    if "nc" not in _CACHE:
        _CACHE["nc"] = _build_nc()
    return _CACHE["nc"]


def _in_maps(x):
    ident = np.eye(128, dtype=np.float32)
    in_maps = []
    for core in range(N_CORES):
        b, qh = core // 2, core % 2
        x0b = x[b, :, :D_HALF]
        if qh:
            x0p = np.ascontiguousarray(np.roll(x0b, -D_HALF, axis=0))
        else:
            x0p = np.ascontiguousarray(x0b)
        x1q = np.ascontiguousarray(x[b, qh * D_HALF:(qh + 1) * D_HALF, D_HALF:])
        in_maps.append({"x0": x0p, "x1": x1q, "ident": ident})
    return in_maps


def kernel(x):
    from concourse.bass_utils import run_bass_kernel_spmd

    x = np.asarray(x)
    assert x.shape == (4, T, 2 * D_HALF), x.shape
    x = np.ascontiguousarray(x, dtype=np.float32)

    nc = _get_nc()
    res = run_bass_kernel_spmd(nc, _in_maps(x), core_ids=list(range(N_CORES)))

    out = np.empty((4, T, D_HALF), dtype=np.float32)
    for core in range(N_CORES):
        b, qh = core // 2, core % 2
        out[b, qh * D_HALF:(qh + 1) * D_HALF, :] = res.results[core]["out"]
    return out
